# revision 1
# baseline (speedup 1.0000x reference)
"""AutoCorrelation kernel for Trainium2 (Bass/Tile), 8-core data parallel.

Math: the reference computes rfft over the zero-padded head dim (D=64 -> L=512),
multiplies conj(Q)*K, irffts, then MEANS over heads AND the whole lag axis.
Summing a circular correlation over all lags factorizes:
    sum_t corr[t] = (sum_d q[d]) * (sum_d k[d])
so  x_corr_mean[b,l] = 1/(H*L) * sum_h (sum_d q[b,l,h,:]) * (sum_d k[b,l,h,:]).
Then top-6 over l per batch, softmax, weighted sum of values rows -> [B,H,D].

Sharding: batch 16 -> 2 per core across 8 cores, no cross-core communication.

Per core: per batch, q/k row-sums + per-head products on DVE while HWDGE DMAs
stream in (k split 3+1 chunks so the last reduce is short); PE-transpose +
ACT copy + small DMA rake corr into corr2[b, :] (all off-DVE so batch 0's
stretch doesn't contend with batch 1's reduces).  Tail: one MAX8/FIND_INDEX8
pair for both batches, per-batch indirect gathers fed straight from the
FIND_INDEX8 output row (batch base via element_offset), softmax weights
stream-transposed once, per-batch tiny matmuls, stores.
"""

import numpy as np

import concourse.bass as bass
import concourse.mybir as mybir
import concourse.tile as tile
from concourse.masks import make_identity
from concourse.bass_utils import run_bass_kernel_spmd

B, L, H, D = 16, 512, 8, 64
HD = H * D                  # 512
NCORES = 8
BPC = B // NCORES           # 2 batches per core
ROWS = BPC * L              # 1024 rows of [HD] per core
P = 128
NT = ROWS // P              # 8 chunks of 128 rows
TPB = L // P                # 4 chunks per batch
KTOP = 6                    # k = int(log(512)) = 6
SCALE = 1.0 / (H * L)

_CACHE = {}


def _emit(tc, q, k, v, out):
    # out: single [BPC, HD] DRAM AP.
    nc = tc.nc
    from contextlib import ExitStack

    with ExitStack() as ctx:
        main = ctx.enter_context(tc.tile_pool(name="main", bufs=2))
        small = ctx.enter_context(tc.tile_pool(name="small", bufs=1))
        psum = ctx.enter_context(tc.tile_pool(name="psum", bufs=1, space="PSUM"))

        ident = small.tile([P, P], mybir.dt.float32)
        make_identity(nc, ident[:])

        # Per-partition batch masks m0=[1,0], m1=[0,1] (built once, early;
        # 0/1 are exact in f32 so iota can emit float directly).
        m1f = small.tile([BPC, 1], mybir.dt.float32)
        nc.gpsimd.iota(
            m1f[:], pattern=[[0, 1]], base=0, channel_multiplier=1,
            allow_small_or_imprecise_dtypes=True,
        )
        m0f = small.tile([BPC, 1], mybir.dt.float32)
        nc.gpsimd.iota(
            m0f[:], pattern=[[0, 1]], base=1, channel_multiplier=-1,
            allow_small_or_imprecise_dtypes=True,
        )
        mLf = small.tile([BPC, 1], mybir.dt.float32)
        nc.vector.tensor_scalar_mul(mLf[:], m1f[:], float(L))

        q3 = q.rearrange("(t p) m -> t p m", p=P)
        k3 = k.rearrange("(t p) m -> t p m", p=P)

        corr2 = small.tile([BPC, L], mybir.dt.float32)
        for b in range(BPC):
            t0 = b * TPB

            # q split 2+2 chunks, k split 2+1+1: reduces pipeline against
            # DMA arrival and the serial work after the last byte lands is
            # one short 0.25MB reduce instead of a monolithic 1MB one.
            qt = main.tile([P, TPB, HD], mybir.dt.float32, tag=f"qt{b}")
            kt = main.tile([P, TPB, HD], mybir.dt.float32, tag=f"kt{b}")
            q_pieces = [(0, 2), (2, 4)]
            k_pieces = [(0, 2), (2, 3), (3, 4)]
            for lo, hi in q_pieces:
                nc.sync.dma_start(
                    out=qt[:, lo:hi, :],
                    in_=q3[t0 + lo : t0 + hi].rearrange("t p m -> p t m"),
                )
            for lo, hi in k_pieces:
                nc.sync.dma_start(
                    out=kt[:, lo:hi, :],
                    in_=k3[t0 + lo : t0 + hi].rearrange("t p m -> p t m"),
                )
            sq = small.tile([P, TPB * H], mybir.dt.float32, tag=f"sq{b}")
            sk = small.tile([P, TPB * H], mybir.dt.float32, tag=f"sk{b}")
            for lo, hi in q_pieces:
                nc.vector.reduce_sum(
                    out=sq[:, lo * H : hi * H],
                    in_=qt[:, lo:hi, :].rearrange("p t (h d) -> p (t h) d", d=D),
                    axis=mybir.AxisListType.X,
                )
            for lo, hi in k_pieces:
                nc.vector.reduce_sum(
                    out=sk[:, lo * H : hi * H],
                    in_=kt[:, lo:hi, :].rearrange("p t (h d) -> p (t h) d", d=D),
                    axis=mybir.AxisListType.X,
                )
            prod = small.tile([P, TPB * H], mybir.dt.float32, tag=f"prod{b}")
            nc.vector.tensor_mul(prod[:], sq[:], sk[:])
            corr_b = small.tile([P, TPB], mybir.dt.float32, tag=f"corr{b}")
            nc.vector.reduce_sum(
                out=corr_b[:],
                in_=prod[:].rearrange("p (t h) -> p t h", h=H),
                axis=mybir.AxisListType.X,
            )

            # corr [128, 4] -> [4, 128] on PE, ACT copies it out of PSUM,
            # one small DMA rakes it into corr2[b, :].
            psumT = psum.tile([TPB, P], mybir.dt.float32, tag=f"psumT{b}")
            nc.tensor.transpose(out=psumT[:], in_=corr_b[:], identity=ident[:])
            corrT = small.tile([TPB, P], mybir.dt.float32, tag=f"corrT{b}")
            nc.scalar.copy(corrT[:], psumT[:])
            nc.sync.dma_start(out=corr2[b : b + 1, :], in_=corrT[:])

        # ---- tail ----
        maxv = small.tile([BPC, 8], mybir.dt.float32)
        maxi = small.tile([BPC, 8], mybir.dt.uint32)
        nc.vector.max(out=maxv[:], in_=corr2[:])
        nc.vector.max_index(out=maxi[:], in_max=maxv[:], in_values=corr2[:])

        # Combined 12-row gather index column (float staging; indices are
        # exact below 2^24): stage_i row 0 cols 0:6 = idx_b0, row 1 cols
        # 6:12 = idx_b1 + L via the masks; after the 32x32 transpose,
        # col0+col1 rows 0:12 = [idx0, idx1 + L] -> one indirect gather.
        idxf = small.tile([BPC, 8], mybir.dt.float32)
        nc.vector.tensor_copy(idxf[:], maxi[:])
        stage_i = small.tile([32, 32], mybir.dt.float32)
        nc.vector.memset(stage_i[:], 0.0)
        nc.vector.tensor_scalar(
            stage_i[0:BPC, 0:KTOP],
            idxf[:, 0:KTOP],
            m0f[:, 0:1],
            scalar2=None,
            op0=mybir.AluOpType.mult,
        )
        nc.vector.tensor_scalar(
            stage_i[0:BPC, KTOP : 2 * KTOP],
            idxf[:, 0:KTOP],
            mLf[:, 0:1],
            scalar2=m1f[:, 0:1],
            op0=mybir.AluOpType.add,
            op1=mybir.AluOpType.mult,
        )
        stageT_i = small.tile([32, 32], mybir.dt.float32)
        nc.vector.transpose(out=stageT_i[:], in_=stage_i[:])
        combf = small.tile([32, 1], mybir.dt.float32)
        nc.vector.tensor_add(combf[:], stageT_i[:, 0:1], stageT_i[:, 1:2])
        comb = small.tile([32, 1], mybir.dt.uint32)
        nc.vector.tensor_copy(comb[:], combf[:])

        # softmax over the top-6 of corr*SCALE (|corr*SCALE| < ~1, so
        # skipping the max-subtraction is safe in fp32); weights hop onto
        # partitions via one 32x32 stream transpose.
        e = small.tile([BPC, KTOP], mybir.dt.float32)
        nc.scalar.activation(
            out=e[:],
            in_=maxv[:, 0:KTOP],
            func=mybir.ActivationFunctionType.Exp,
            scale=SCALE,
        )
        s = small.tile([BPC, 1], mybir.dt.float32)
        nc.vector.reduce_sum(out=s[:], in_=e[:], axis=mybir.AxisListType.X)
        rs = small.tile([BPC, 1], mybir.dt.float32)
        nc.vector.reciprocal(out=rs[:], in_=s[:])
        w = small.tile([BPC, KTOP], mybir.dt.float32)
        nc.vector.tensor_scalar_mul(w[:], e[:], rs[:, 0:1])

        # Block-diagonal [12, 2] weights via the mask trick: stage_w[b, 0:6]
        # = w_b0 masked to row 0, stage_w[b, 6:12] = w_b1 masked to row 1;
        # transpose -> stageT_w[0:6, 0] = w0, stageT_w[6:12, 1] = w1.
        stage_w = small.tile([32, 32], mybir.dt.float32)
        nc.vector.memset(stage_w[:], 0.0)
        nc.vector.tensor_scalar(
            stage_w[0:BPC, 0:KTOP],
            w[:],
            m0f[:, 0:1],
            scalar2=None,
            op0=mybir.AluOpType.mult,
        )
        nc.vector.tensor_scalar(
            stage_w[0:BPC, KTOP : 2 * KTOP],
            w[:],
            m1f[:, 0:1],
            scalar2=None,
            op0=mybir.AluOpType.mult,
        )
        stageT_w = small.tile([32, 32], mybir.dt.float32)
        nc.vector.transpose(out=stageT_w[:], in_=stage_w[:])

        # ONE indirect gather of all 12 selected value rows, then weighted-sum
        # both batches with ONE matmul, one copy, one store.
        gath = small.tile([2 * KTOP, HD], mybir.dt.float32)
        nc.gpsimd.indirect_dma_start(
            out=gath[:],
            out_offset=None,
            in_=v,
            in_offset=bass.IndirectOffsetOnAxis(
                ap=comb[0 : 2 * KTOP, 0:1], axis=0
            ),
        )
        acc = psum.tile([BPC, HD], mybir.dt.float32)
        nc.tensor.matmul(
            out=acc[:],
            lhsT=stageT_w[0 : 2 * KTOP, 0:BPC],
            rhs=gath[:],
            start=True,
            stop=True,
        )
        outt = small.tile([BPC, HD], mybir.dt.float32)
        nc.scalar.copy(outt[:], acc[:])
        nc.sync.dma_start(out=out, in_=outt[:])


def _build_bass():
    import concourse.bacc as bacc

    nc = bacc.Bacc(trn_type="TRN2", target_bir_lowering=False, debug=False)
    q = nc.dram_tensor("q", [ROWS, HD], mybir.dt.float32, kind="ExternalInput").ap()
    k = nc.dram_tensor("k", [ROWS, HD], mybir.dt.float32, kind="ExternalInput").ap()
    v = nc.dram_tensor("v", [ROWS, HD], mybir.dt.float32, kind="ExternalInput").ap()
    out = nc.dram_tensor(
        "out", [BPC, HD], mybir.dt.float32, kind="ExternalOutput"
    ).ap()
    with tile.TileContext(nc) as tc:
        _emit(tc, q, k, v, out)
    nc.compile()
    return nc


def _get_nc():
    if "nc" not in _CACHE:
        _CACHE["nc"] = _build_bass()
    return _CACHE["nc"]


def run_sharded(queries, keys, values, trace=False, **kw):
    """Shard over 8 cores, run, gather. Returns (out [16,8,64], BassKernelResults)."""
    nc = _get_nc()
    q = np.ascontiguousarray(np.asarray(queries, dtype=np.float32))
    k = np.ascontiguousarray(np.asarray(keys, dtype=np.float32))
    v = np.ascontiguousarray(np.asarray(values, dtype=np.float32))
    in_maps = []
    for c in range(NCORES):
        sl = slice(c * BPC, (c + 1) * BPC)
        in_maps.append(
            {
                "q": q[sl].reshape(ROWS, HD),
                "k": k[sl].reshape(ROWS, HD),
                "v": v[sl].reshape(ROWS, HD),
            }
        )
    res = run_bass_kernel_spmd(nc, in_maps, list(range(NCORES)), trace=trace, **kw)
    out = np.empty((B, H, D), dtype=np.float32)
    for c in range(NCORES):
        out[c * BPC : (c + 1) * BPC] = res.results[c]["out"].reshape(BPC, H, D)
    return out, res


def kernel(queries, keys, values, B=None, **_ignored):
    out, _ = run_sharded(queries, keys, values, trace=False)
    return out

